# revision 1
# baseline (speedup 1.0000x reference)
"""Self-contained Trainium2 Bass kernel for nn_MultiHeadAttention_50800873177468.

B=8, T=1024, D=1024, H=16 causal MHA (Whisper-style). Data-parallel over
batch: core c computes batch c. Returns (out, qk) like the reference.

Per-core dataflow (all matmuls f32r or bf16, fp32 PSUM accumulate):
  phase 1: qT = Wq@xT + bq, kT = 0.125*(Wk@xT), v = x@WvT + bv
  phase 2: per (head, tq-tile): s = qT.T@kT (K=64), +mask on diag block,
           qk out <- s (ACT copy), w = exp(s) bf16 + row-sum (ACT accum),
           w *= 1/sum (DVE), wT = PE-transpose(w), wv^T += v.T@wT
  phase 3: out = wv@WoT + bo
Host: input transposes, -inf fill of the qk upper triangle, stacking.
"""
import os
import sys
import types

for _p in ("/opt/trn_rl_repo", "/root/.axon_site/_ro/trn_rl_repo"):
    if os.path.isdir(_p) and _p not in sys.path:
        sys.path.append(_p)

import numpy as np

# NTFF profile hook shim (missing antenv.axon_hooks in this image)
if 'antenv.axon_hooks' not in sys.modules:
    _m = types.ModuleType('antenv.axon_hooks')
    _h = [None]
    _m.get_axon_ntff_profile_hook = lambda: _h[0]
    _m.set_axon_ntff_profile_hook = lambda h: _h.__setitem__(0, h)
    sys.modules['antenv.axon_hooks'] = _m
    try:
        from trn_agent_boot.trn_boot import _ntff_profile_via_ctypes
        _m.set_axon_ntff_profile_hook(
            _ntff_profile_via_ctypes('/opt/axon/libaxon_pjrt.so'))
    except Exception:
        pass

import concourse.bass as bass
import concourse.tile as tile
from concourse import bacc, mybir
from concourse import bass_utils
from concourse.bass_interp import get_hw_module
from concourse.masks import make_identity

f32 = mybir.dt.float32
f32r = mybir.dt.float32r
bf16 = mybir.dt.bfloat16
EXP = mybir.ActivationFunctionType.Exp
ADD = mybir.AluOpType.add

N_CORES = 8
B, T, D, H = 8, 1024, 1024, 16
HD = D // H            # 64
P = 128
NT = T // P            # 8 tq tiles
ND = D // P            # 8 d tiles
S2 = 0.125             # (hd ** -0.25) ** 2, exact in fp32

TRACE = bool(os.environ.get("BASS_KERNEL_TRACE"))
_CACHE = {}


def _emit(nc, tc, ap):
    ctx_pools = []

    def pool(name, **kw):
        p = tc.tile_pool(name=name, **kw)
        return p

    with pool("persist", bufs=1) as persist:
        # small persistent operands
        bqs = persist.tile([P, ND], f32)          # bq as per-partition cols
        nc.sync.dma_start(out=bqs, in_=ap["bq"].rearrange("(g p) -> p g", p=P))
        maskd = persist.tile([P, P], f32)
        nc.sync.dma_start(out=maskd, in_=ap["maskd"])
        bvb = persist.tile([P, D], bf16)          # bv broadcast along partitions
        nc.gpsimd.dma_start(out=bvb, in_=bass.AP(
            tensor=ap["bv"].tensor, offset=ap["bv"].offset,
            ap=[[0, P], [1, D]]))
        bob = persist.tile([P, D], f32)           # bo broadcast
        nc.gpsimd.dma_start(out=bob, in_=bass.AP(
            tensor=ap["bo"].tensor, offset=ap["bo"].offset,
            ap=[[0, P], [1, D]]))
        ident = persist.tile([P, P], bf16)
        make_identity(nc, ident)

        with pool("qkv", bufs=1) as qkv:
            qT = qkv.tile([P, ND, T], f32r)       # q^T  [dout, t]
            kT = qkv.tile([P, ND, T], f32r)       # k^T * 0.125
            vN = qkv.tile([P, NT, D], bf16)       # v natural [t, d]
            wvT = qkv.tile([P, ND, T], bf16)      # (w@v)^T  [dj, tq]

            # ---------------- phase 1: projections ----------------
            with pool("ph1", bufs=1) as ph1, \
                 pool("ph1w", bufs=1) as ph1w, \
                 pool("ps1", bufs=4, space="PSUM") as ps1:
                xT = ph1.tile([P, ND, T], f32r)
                nc.gpsimd.dma_start(
                    out=xT, in_=ap["x"].rearrange("(g p) t -> p g t", p=P))

                for wname, scale in (("WqT", None), ("WkT", S2)):
                    wt = ph1w.tile([P, ND, D], f32r, tag="w")
                    nc.gpsimd.dma_start(
                        out=wt, in_=ap[wname].rearrange("(g p) d -> p g d", p=P))
                    dst = qT if wname == "WqT" else kT
                    for g in range(ND):          # dout tile
                        for c in range(2):       # t chunk of 512
                            ps = ps1.tile([P, 512], f32, tag="ps")
                            for kk in range(ND):
                                nc.tensor.matmul(
                                    ps, wt[:, kk, g * P:(g + 1) * P],
                                    xT[:, kk, c * 512:(c + 1) * 512],
                                    start=(kk == 0), stop=(kk == ND - 1))
                            o = dst[:, g, c * 512:(c + 1) * 512]
                            if scale is None:
                                nc.vector.tensor_scalar_add(o, ps, bqs[:, g:g + 1])
                            else:
                                nc.vector.tensor_scalar_mul(o, ps, scale)

                wt = ph1w.tile([P, ND, D], f32r, tag="w")
                nc.gpsimd.dma_start(
                    out=wt, in_=ap["WvT"].rearrange("(g p) d -> p g d", p=P))
                for tt in range(NT):
                    for c in range(2):
                        ps = ps1.tile([P, 512], f32, tag="ps")
                        for kk in range(ND):
                            nc.tensor.matmul(
                                ps, xT[:, kk, tt * P:(tt + 1) * P],
                                wt[:, kk, c * 512:(c + 1) * 512],
                                start=(kk == 0), stop=(kk == ND - 1))
                        o = vN[:, tt, c * 512:(c + 1) * 512]
                        nc.vector.tensor_copy(o, ps)
                        nc.vector.tensor_tensor(
                            out=o, in0=o, in1=bvb[:, c * 512:(c + 1) * 512], op=ADD)

            # ---------------- phase 2: attention ----------------
            with pool("ph2", bufs=2) as ph2, \
                 pool("ph2s", bufs=3) as ph2s, \
                 pool("ps2", bufs=2, space="PSUM") as ps2, \
                 pool("ps2t", bufs=2, space="PSUM") as ps2t, \
                 pool("ps2w", bufs=2, space="PSUM") as ps2w:
                for h in range(H):
                    g, ro = h // 2, (h % 2) * HD
                    wTb = ph2.tile([P, NT, T], bf16, tag="wT")
                    for i in range(NT):
                        span = (i + 1) * P
                        sps = ps2.tile([P, T], f32, tag="s")
                        for c0 in range(0, span, 512):
                            c1 = min(c0 + 512, span)
                            nc.tensor.matmul(
                                sps[:, c0:c1],
                                qT[ro:ro + HD, g, i * P:(i + 1) * P],
                                kT[ro:ro + HD, g, c0:c1],
                                start=True, stop=True)
                        # causal mask on the diagonal block
                        nc.vector.tensor_tensor(
                            out=sps[:, i * P:span], in0=sps[:, i * P:span],
                            in1=maskd, op=ADD)
                        # qk output (masked scores); host fills the upper triangle
                        qkst = ph2s.tile([P, T], f32, tag="qkst")
                        nc.scalar.copy(qkst[:, :span], sps[:, :span])
                        nc.sync.dma_start(
                            out=ap["qk"][h, i * P:(i + 1) * P, 0:span],
                            in_=qkst[:, :span])
                        # softmax numerator + row sum
                        wt_ = ph2s.tile([P, T], bf16, tag="wtile")
                        lsum = ph2s.tile([P, 1], f32, tag="lsum")
                        nc.scalar.activation(
                            wt_[:, :span], sps[:, :span], EXP, accum_out=lsum)
                        rsum = ph2s.tile([P, 1], f32, tag="rsum")
                        nc.vector.reciprocal(rsum, lsum)
                        nc.vector.tensor_scalar_mul(
                            wt_[:, :span], wt_[:, :span], rsum)
                        # transpose w tiles: wT[tk, tq] per tk-tile
                        tps = ps2t.tile([P, T], bf16, tag="tp")
                        for t in range(i + 1):
                            nc.tensor.matmul(
                                tps[:, t * P:(t + 1) * P],
                                wt_[:, t * P:(t + 1) * P], ident,
                                is_transpose=True)
                        nc.vector.tensor_copy(
                            wTb[:, 0:i + 1, i * P:(i + 1) * P], tps[:, :span])
                    # wv^T accumulation for this head
                    for c in range(2):
                        wps = ps2w.tile([HD, 512], f32, tag="wv")
                        nmm = 4 * c + 4
                        for t in range(nmm):
                            cs = max(c * 512, t * P)
                            nc.tensor.matmul(
                                wps[:, cs - c * 512:512],
                                vN[:, t, h * HD:(h + 1) * HD],
                                wTb[:, t, cs:(c + 1) * 512],
                                start=(t == 0), stop=(t == nmm - 1))
                        nc.vector.tensor_copy(
                            wvT[ro:ro + HD, g, c * 512:(c + 1) * 512], wps)

        # ---------------- phase 3: output projection ----------------
        with pool("ph3", bufs=1) as ph3, \
             pool("ph3s", bufs=3) as ph3s, \
             pool("ps3", bufs=4, space="PSUM") as ps3:
            wo = ph3.tile([P, ND, D], bf16)
            nc.gpsimd.dma_start(
                out=wo, in_=ap["WoT"].rearrange("(g p) d -> p g d", p=P))
            for it in range(NT):
                ost = ph3s.tile([P, D], f32, tag="ost")
                for c in range(2):
                    ps = ps3.tile([P, 512], f32, tag="ps")
                    for g in range(ND):
                        nc.tensor.matmul(
                            ps, wvT[:, g, it * P:(it + 1) * P],
                            wo[:, g, c * 512:(c + 1) * 512],
                            start=(g == 0), stop=(g == ND - 1))
                    nc.vector.tensor_tensor(
                        out=ost[:, c * 512:(c + 1) * 512], in0=ps,
                        in1=bob[:, c * 512:(c + 1) * 512], op=ADD)
                nc.sync.dma_start(
                    out=ap["out"][it * P:(it + 1) * P, :], in_=ost)


def _build():
    if "nc" in _CACHE:
        return _CACHE["nc"]
    nc = bacc.Bacc("TRN2", target_bir_lowering=False, debug=False,
                   enable_asserts=False, num_devices=N_CORES)
    ap = {}
    for name, shape in (("x", [D, T]), ("WqT", [D, D]), ("WkT", [D, D]),
                        ("WvT", [D, D]), ("WoT", [D, D]), ("bq", [D]),
                        ("bv", [D]), ("bo", [D]), ("maskd", [P, P])):
        ap[name] = nc.dram_tensor(name, shape, f32, kind="ExternalInput").ap()
    ap["out"] = nc.dram_tensor("out", [T, D], f32, kind="ExternalOutput").ap()
    ap["qk"] = nc.dram_tensor("qk", [H, T, T], f32, kind="ExternalOutput").ap()

    with tile.TileContext(nc) as tc:
        _emit(nc, tc, ap)
    nc.compile()
    nc.m = get_hw_module(nc.m)
    _CACHE["nc"] = nc
    return nc


def kernel(x, mask, Wq, bq, Wk, Wv, bv, Wo, bo):
    nc = _build()
    x = np.ascontiguousarray(x, dtype=np.float32)
    base = {
        "WqT": np.ascontiguousarray(Wq.T, dtype=np.float32),
        "WkT": np.ascontiguousarray(Wk.T, dtype=np.float32),
        "WvT": np.ascontiguousarray(Wv.T, dtype=np.float32),
        "WoT": np.ascontiguousarray(Wo.T, dtype=np.float32),
        "bq": np.ascontiguousarray(bq, dtype=np.float32),
        "bv": np.ascontiguousarray(bv, dtype=np.float32),
        "bo": np.ascontiguousarray(bo, dtype=np.float32),
        "maskd": np.ascontiguousarray(mask[:P, :P], dtype=np.float32),
    }
    in_maps = [dict(base, x=np.ascontiguousarray(x[c].T)) for c in range(B)]

    res = bass_utils.run_bass_kernel_spmd(
        nc, in_maps, core_ids=list(range(N_CORES)), trace=TRACE)
    if TRACE:
        _CACHE["last_results"] = res

    out = np.stack([res.results[c]["out"] for c in range(B)])
    qk = np.stack([res.results[c]["qk"] for c in range(B)])
    triu = np.triu(np.ones((T, T), dtype=bool), k=1)
    qk[:, :, triu] = -np.inf
    return out, qk
